# revision 9
# baseline (speedup 1.0000x reference)
"""Haar wavelet (2x2 stride-2, per-channel) Trainium2 Bass kernel.

Full input x: (8, 64, 512, 512) f32 -> full output (8, 256, 256, 256) f32.
Sharding: pure data parallel over batch -- core i processes x[i].

Per-core design (v6), built from microbenchmark findings:
  * Solo (serialized) DMAs sustain ~350 GB/s in either direction for any
    descriptor shape; concurrent read+write traffic collapses to ~300.
    So DMAs are issued on ONE HWDGE ring (SP) in alternating
    store-phase / load-phase order, with loads serialized by a bufs=1
    input tile (load b+1 WAW-waits the ACT cast of block b).  Engines
    then run direction-pure ~4MiB phases back to back.
  * Layout: partition p = h*64 + c (c = channel, h = row-half of the
    block).  Block b covers input rows [b*32, (b+1)*32): one contiguous
    32KB DRAM run per partition per load; output = 4 runs of 8KB per
    partition per store (4 subband channels x 8 consecutive out rows).
  * Compute: ACT casts f32 -> bf16 with the x0.5 folded into the
    activation scale; DVE does the vertical butterfly in bf16 (2x mode)
    and the horizontal butterfly bf16 -> f32 (1x).  DVE ~220us total,
    under the ~375us DMA span floor.  bf16 rounding gives rel err
    ~4e-3, within the 2e-2 gate.
"""

import sys

if "/opt/trn_rl_repo" not in sys.path:
    sys.path.insert(0, "/opt/trn_rl_repo")

from contextlib import ExitStack

import numpy as np

import concourse.bass as bass
import concourse.tile as tile
from concourse import bacc
from concourse import mybir
from concourse.bass_utils import run_bass_kernel_spmd

N_CORES = 8
C, H, W = 64, 512, 512
F32 = mybir.dt.float32
BF16 = mybir.dt.bfloat16
ADD = mybir.AluOpType.add
SUB = mybir.AluOpType.subtract

_CACHED = {}


def _build(C=C, H=H, W=W, R=8):
    HO, WO = H // 2, W // 2
    NB = H // (4 * R)  # 16 blocks of 4R=32 input rows
    nc = bacc.Bacc("TRN2", target_bir_lowering=False, debug=False)
    x = nc.dram_tensor("x", [C, H, W], F32, kind="ExternalInput").ap()
    out = nc.dram_tensor("out", [4 * C, HO, WO], F32, kind="ExternalOutput").ap()
    out4 = out.rearrange("(c q) ho w -> c q ho w", q=4)

    with tile.TileContext(nc) as tc, ExitStack() as ctx:
        xpool = ctx.enter_context(tc.tile_pool(name="xp", bufs=1))
        hpool = ctx.enter_context(tc.tile_pool(name="hp", bufs=2))
        mpool = ctx.enter_context(tc.tile_pool(name="mid", bufs=2))
        rpool = ctx.enter_context(tc.tile_pool(name="res", bufs=2))

        def emit_load(b):
            xt = xpool.tile([128, 2 * R * W], F32)
            for h in (0, 1):
                src = x[
                    :, b * 4 * R + h * 2 * R : b * 4 * R + (h + 1) * 2 * R, :
                ].rearrange("c t w -> c (t w)")
                nc.sync.dma_start(xt[h * 64 : (h + 1) * 64, :], src)
            return xt

        def emit_stage1(xt):
            # cast f32 -> bf16 with the Haar x0.5 folded into the scale.
            # ACT is xt's only reader, so the next load's WAW dep clears
            # right after this op -- the load chain never waits on DVE.
            xh = hpool.tile([128, 2 * R * W], BF16)
            nc.scalar.mul(xh[:], xt[:], 0.5)
            x4 = xh[:].rearrange("p (r t w) -> p r t w", t=2, w=W)
            top, bot = x4[:, :, 0, :], x4[:, :, 1, :]
            s_t = mpool.tile([128, R * W], BF16, tag="s")
            d_t = mpool.tile([128, R * W], BF16, tag="d")
            sv = s_t[:].rearrange("p (r w) -> p r w", w=W)
            dv = d_t[:].rearrange("p (r w) -> p r w", w=W)
            nc.vector.tensor_tensor(sv, top, bot, ADD)
            nc.vector.tensor_tensor(dv, bot, top, SUB)
            return s_t, d_t

        def emit_stage2(b, s_t, d_t):
            s2 = s_t[:].rearrange("p (r j t) -> p r j t", t=2, j=WO)
            d2 = d_t[:].rearrange("p (r j t) -> p r j t", t=2, j=WO)
            s_e, s_o = s2[:, :, :, 0], s2[:, :, :, 1]
            d_e, d_o = d2[:, :, :, 0], d2[:, :, :, 1]
            rt = rpool.tile([128, 4 * R * WO], F32)
            r4 = rt[:].rearrange("p (q r j) -> p q r j", q=4, j=WO)
            nc.vector.tensor_tensor(r4[:, 0], s_e, s_o, ADD)  # ll
            nc.vector.tensor_tensor(r4[:, 1], d_e, d_o, ADD)  # lh
            nc.vector.tensor_tensor(r4[:, 2], s_o, s_e, SUB)  # hl
            nc.vector.tensor_tensor(r4[:, 3], d_o, d_e, SUB)  # hh
            for h in (0, 1):
                dst = out4[:, :, b * 2 * R + h * R : b * 2 * R + (h + 1) * R, :]
                nc.sync.dma_start(dst, r4[h * 64 : (h + 1) * 64])

        # Emission order per iteration: stage2(b-1) FIRST so horiz(b-1)
        # precedes vert(b) in DVE program order -- the store chain then
        # never waits on the current load, and the SP ring alternates
        # S(b-1), L(b+1) with no head-of-line stalls.
        xt = emit_load(0)
        pending = None  # (b, s_t, d_t) awaiting stage2 + store
        for b in range(NB):
            if pending is not None:
                emit_stage2(*pending)  # horiz(b-1) + stores -> ring
            s_t, d_t = emit_stage1(xt)
            if b + 1 < NB:
                xt = emit_load(b + 1)  # loads queue behind the stores
            pending = (b, s_t, d_t)
        emit_stage2(*pending)
    nc.compile()
    return nc


def _get_nc():
    if "nc" not in _CACHED:
        _CACHED["nc"] = _build()
    return _CACHED["nc"]


def _run(x, **kwargs):
    x = np.ascontiguousarray(np.asarray(x), dtype=np.float32)
    assert x.shape == (N_CORES, C, H, W), x.shape
    nc = _get_nc()
    in_maps = [{"x": np.ascontiguousarray(x[i])} for i in range(N_CORES)]
    res = run_bass_kernel_spmd(nc, in_maps, core_ids=list(range(N_CORES)), **kwargs)
    out = np.stack([res.results[i]["out"] for i in range(N_CORES)], axis=0)
    return out, res


def kernel(x):
    return _run(x)[0]
